# revision 26
# baseline (speedup 1.0000x reference)
"""Trainium2 Bass kernel for nn_BaseConvPlus (dense_cnn).

Math: the reference computes
  1) kernel[b,c,:,:]  = global-mean of a depthwise 3x3 conv of x          -> [B,CIN,3,3]
  2) win  = einsum(kernel, w_in) + b_in ; wout = einsum(kernel, w_out)
  3) y[b] = conv2d(x[b], weight[b]) with weight[b,o,i] = win[b,i]*wout[b,o]

Identities / tricks:
  * mean(conv(x, k)) over HxW only needs the total sum, edge-row/col sums
    and corner pixels of each channel (zero 'SAME' padding) - no conv.
    The tap-selection matrix is folded into the host-side wk tables, so
    kernel[b,c,j] = sum_k wkH[c,j,k] * sums[b,c,k] with sums = the 9
    reduced quantities [T, CF, CL, RF, RL, c00, c0L, cL0, cLL].
  * weight[b] is rank-1 across (o, i): y[b,o] = wout[b,o] * z[b] with
    z[b] = sum_i conv2d(x[b,i], win[b,i]).
  * x arrives host-padded (zero ring) and bf16, so all 9 (ky,kx) taps are
    plain shifted windows of the padded image.  Stage 1 runs the 9 tap
    matmuls (M=4 real outputs each) as 3 accumulation chains on 3
    concurrent PE *column groups* (tile_position col packing), so the 9
    image passes cost ~3.  One [96,384] eviction per tile feeds stage 2,
    a single K=96 matmul that applies wout (1 pass).  ~4 effective
    passes/tile vs 6 in the two-stage K=128/K=12 formulation, and the
    eviction count drops 1.5x.
  * bf16 end to end (input pre-cast on host, output upcast on host):
    halves both HBM phases.  Dummy matmuls paced by the input chunks keep
    the PE HAM warm (2.4 GHz) through the load phase.

Sharding: pure data parallel, 4 samples per core on 8 cores.
"""
import sys

sys.path.insert(0, "/opt/trn_rl_repo")

from contextlib import ExitStack

import ml_dtypes
import numpy as np

import concourse.bacc as bacc
import concourse.bass as bass
import concourse.mybir as mybir
import concourse.tile as tile
from concourse.bass_utils import run_bass_kernel_spmd

B, CIN, COUT, KS, H, W = 32, 32, 32, 3, 192, 192
NCORES = 8
BC = B // NCORES          # 4 samples per core
P = BC * CIN              # 128 partitions = (sample, channel)
WP = W + 2                # 194 padded cols
HP = H + 2                # 194 padded rows
R = 2                     # output rows per conv tile
NT = H // R               # 96 conv tiles
GT = 8                    # conv tiles per output DMA (16 rows)
NG = NT // GT             # 12 output DMAs
N = R * W                 # 384 moving columns per matmul
CHUNKS = [16] * 9 + [10] * 4 + [5, 5]   # input chunks over the 194 padded rows
NCHUNK = len(CHUNKS)      # 15
F32 = mybir.dt.float32
BF16 = mybir.dt.bfloat16
AX = mybir.AxisListType
OP = mybir.AluOpType


def build_program(nc: bass.Bass) -> None:
    x_d = nc.dram_tensor("xpad", [P, HP * WP], BF16, kind="ExternalInput").ap()
    wkh_d = nc.dram_tensor("wkh", [P, 81], F32, kind="ExternalInput").ap()
    lwin_d = nc.dram_tensor("lwin", [P, P], BF16, kind="ExternalInput").ap()
    brep_d = nc.dram_tensor("brep", [P, 1], F32, kind="ExternalInput").ap()
    wo9_d = nc.dram_tensor("wo9", [P, 9 * P], BF16, kind="ExternalInput").ap()
    m4_d = nc.dram_tensor("m4", [P, 32], F32, kind="ExternalInput").ap()
    ident_d = nc.dram_tensor("ident", [P, P], BF16, kind="ExternalInput").ap()
    y_d = nc.dram_tensor("y", [BC, COUT, H, W], BF16, kind="ExternalOutput").ap()

    yf = y_d.rearrange("b o h w -> (b o) (h w)")       # [128, 36864] bf16

    with tile.TileContext(nc) as tc, ExitStack() as ctx:
        const = ctx.enter_context(tc.tile_pool(name="const", bufs=1))
        ypool = ctx.enter_context(tc.tile_pool(name="ysb", bufs=3))
        psum_z = ctx.enter_context(tc.tile_pool(name="psum_z", bufs=4, space="PSUM"))
        psum_y = ctx.enter_context(tc.tile_pool(name="psum_y", bufs=2, space="PSUM"))

        xpad = const.tile([P, HP * WP], BF16)
        wkh = const.tile([P, 81], F32)
        lwin = const.tile([P, P], BF16)
        brep = const.tile([P, 1], F32)
        wo9 = const.tile([P, 9 * P], BF16)
        m4 = const.tile([P, 32], F32)
        ident = const.tile([P, P], BF16)
        scr = const.tile([P, 16 + 3 * NCHUNK], F32)
        t81 = const.tile([P, 81], F32)
        kern = const.tile([P, 9], F32)
        kernb = const.tile([P, 9], BF16)
        wS1 = const.tile([P, 9 * 4], BF16)     # stage1 lhsT per (ky,kx)
        w2 = const.tile([P, P], BF16)          # stage2 lhsT (wout block diag)
        e4 = const.tile([P, 4], BF16)
        gbuf = const.tile([P, 4 * N], BF16)    # 4 slots of stage1 output
        fold = const.tile([P, 2 * 1552], F32)  # Pool-engine chunk fold scratch
        rscr = const.tile([P, 16 * WP], BF16)  # ACT accum-reduce trash output

        x3 = xpad[:].rearrange("p (r c) -> p r c", c=WP)   # [128, 194, 194]

        nc.vector.memset(w2[:], 0.0)

        # PE keep-warm target (never read; WAW chain keeps it serialized).
        # Shares the psum_z pool: conv stage1 tiles recycle these buffers.
        pdum_t = psum_z.tile([P, 512], F32, tag="zps", name="pdum")
        pdum = pdum_t[:, 0:512]

        # ---- input phase: chunked DMA + running sums + PE warmup ----
        r0 = 0
        for i, lr in enumerate(CHUNKS):
            sl = xpad[:, r0 * WP:(r0 + lr) * WP]
            nc.sync.dma_start(out=sl, in_=x_d[:, r0 * WP:(r0 + lr) * WP])
            if i == 0:
                # constants on the same HWDGE queue (spread over all 16 DMA
                # engines; the SWDGE queue would serialize them on engine 15
                # and that engine's share of every input chunk behind them)
                nc.sync.dma_start(out=wkh[:], in_=wkh_d)
                nc.sync.dma_start(out=lwin[:], in_=lwin_d)
                nc.sync.dma_start(out=brep[:], in_=brep_d)
                nc.sync.dma_start(out=wo9[:], in_=wo9_d)
                nc.sync.dma_start(out=m4[:], in_=m4_d)
                nc.sync.dma_start(out=ident[:], in_=ident_d)
            s3 = sl.rearrange("p (r c) -> p r c", c=WP)
            # even chunks: Pool folds in half, DVE reduces the folded half
            # (Pool is slow: ~0.6 elem/cycle, so only DVE's share is folded);
            # odd chunks: ACT accum-reduce directly
            if i % 2 == 0 and i != NCHUNK - 1:
                hl = lr * WP // 2
                fsl = fold[:, (i // 2 % 2) * 1552:(i // 2 % 2) * 1552 + hl]
                nc.gpsimd.tensor_add(fsl, sl[:, 0:hl], sl[:, hl:2 * hl])
                nc.vector.reduce_sum(out=scr[:, 16 + i:17 + i], in_=fsl,
                                     axis=AX.X)
            elif i % 2 == 0:
                nc.vector.reduce_sum(out=scr[:, 16 + i:17 + i], in_=sl,
                                     axis=AX.X)
            else:
                nc.scalar.activation(
                    out=rscr[:, 0:lr * WP], in_=sl,
                    func=mybir.ActivationFunctionType.Copy,
                    accum_out=scr[:, 16 + i:17 + i])
            if i == 0:     # first data row + top corners (chunk 0 only)
                nc.vector.reduce_sum(out=scr[:, 3:4], in_=x3[:, 1, :],
                                     axis=AX.X)
                nc.vector.tensor_copy(
                    scr[:, 5:7], xpad[:, WP + 1:WP + W + 1:W - 1])
            # HAM keep-warm: dummy matmuls paced by this chunk's arrival
            ndum = 6 if i < 11 else (3 if i < 13 else 2)
            for k in range(ndum):
                nc.tensor.matmul(
                    pdum[:], lhsT=xpad[:, 0:128],
                    rhs=xpad[:, r0 * WP:r0 * WP + 512],
                    start=True, stop=True)
            r0 += lr

        # remaining edge sums: CF on ACT, CL/RL/bottom corners on DVE
        nc.scalar.activation(
            out=rscr[:, 0:HP], in_=x3[:, :, 1],
            func=mybir.ActivationFunctionType.Copy, accum_out=scr[:, 1:2])
        nc.vector.reduce_sum(out=scr[:, 2:3], in_=x3[:, :, W], axis=AX.X)
        nc.vector.reduce_sum(out=scr[:, 4:5], in_=x3[:, H, :], axis=AX.X)
        nc.vector.tensor_copy(
            scr[:, 7:9], xpad[:, H * WP + 1:H * WP + W + 1:W - 1])
        nc.vector.reduce_sum(out=scr[:, 0:1], in_=scr[:, 16:16 + NCHUNK],
                             axis=AX.X)

        # PE bridge through the weight-derivation chain (no deps: these run
        # back-to-back the moment the paced dummies drain, keeping HAM warm)
        for k in range(12):
            nc.tensor.matmul(pdum[:], lhsT=xpad[:, 0:128],
                             rhs=xpad[:, 0:512], start=True, stop=True)

        # kernel[p, j] = sum_k wkH[p, j*9+k] * sums[p, k]
        sums9 = scr[:, 0:9].unsqueeze(1).broadcast_to([P, 9, 9])
        nc.vector.tensor_mul(
            t81[:].rearrange("p (j m) -> p j m", m=9),
            wkh[:].rearrange("p (j m) -> p j m", m=9), sums9)
        nc.vector.reduce_sum(
            out=kern[:], in_=t81[:].rearrange("p (j m) -> p j m", m=9),
            axis=AX.X)
        nc.vector.tensor_copy(kernb[:], kern[:])

        # weight-build outputs live in psum_z-pool buffers (recycled by
        # the first conv tiles once these reads complete); boot garbage in
        # the never-matmul-written regions is zeroed once
        small = psum_z.tile([P, 512], F32, tag="zps", name="small")
        nc.vector.memset(small[:, 160:512], 0.0)
        zscr1 = psum_z.tile([P, 512], F32, tag="zps", name="zscr1")
        nc.vector.memset(zscr1[:], 0.0)
        zscr2 = psum_z.tile([P, 512], F32, tag="zps", name="zscr2")
        nc.vector.memset(zscr2[:], 0.0)
        win_ps = small[:, 0:9]
        wout_ps = small[:, 9:10]
        et_ps = small[0:4, 32:96].bitcast(BF16)

        # win = blockdiag(w_in.T) @ kernel  (+ b_in added in the stt below)
        nc.tensor.matmul(win_ps, lhsT=lwin[:], rhs=kernb[:],
                         start=True, stop=True)

        # stage1 weights: wS1[(b,i), (j,b')] = (win[b,i,j]+b_in[i]) d(b,b')
        wv = win_ps.unsqueeze(2).broadcast_to([P, 9, 4])
        mv = m4[:, 0:4].unsqueeze(1).broadcast_to([P, 9, 4])
        nc.vector.scalar_tensor_tensor(
            wS1[:].rearrange("p (j c) -> p j c", c=4),
            wv, brep[:], mv, op0=OP.add, op1=OP.mult)

        # wout/W2 build is issued after stage1(0): W2 is first needed by
        # stage2(0), three iterations into the conv, so these matmuls ride
        # behind the first conv tile instead of delaying it
        def build_w2():
            for j in range(9):
                nc.tensor.matmul(
                    wout_ps, lhsT=wo9[:, j * P:(j + 1) * P],
                    rhs=kernb[:, j:j + 1], start=(j == 0), stop=(j == 8))
            # W2[32g+b, (b',o)] = wout[(b',o)] d(b,b')
            nc.vector.tensor_scalar_mul(e4[:], m4[:, 0:4], wout_ps)
            nc.tensor.transpose(et_ps, e4[:], ident[:])
            for g in range(4):
                nc.vector.tensor_copy(w2[32 * g:32 * g + 4, :], et_ps)

        # ---- conv phase ----
        # (gpsimd/Pool cannot read PSUM: evictions alternate DVE <-> ACT)
        ysb_tiles = {}
        ypair = {}
        zcp = [nc.vector.tensor_copy,
               lambda o, i_: nc.scalar.copy(out=o, in_=i_)]

        # 9 tap chains spread (3,2,2,2) over the 4 PE column groups; the
        # heavy slot rotates with t so sustained load is 13/4 streams/group
        CHAINS = [[(0, 0), (1, 0), (2, 0)], [(0, 1), (1, 1)],
                  [(2, 1), (0, 2)], [(1, 2), (2, 2)]]

        def stage1(t):
            rot = t % 4
            z_ps = psum_z.tile([P, 512], F32, tag="zps", name="zps")
            for rnd in range(3):
                for c in (1, 2, 3, 0):
                    if rnd >= len(CHAINS[c]):
                        continue
                    ky, kx = CHAINS[c][rnd]
                    g = (c + rot) % 4
                    nc.tensor.matmul(
                        z_ps[32 * g:32 * g + 4, 0:N],
                        lhsT=wS1[:, (3 * ky + kx) * 4:(3 * ky + kx + 1) * 4],
                        rhs=x3[:, 2 * t + ky:2 * t + ky + 2, kx:kx + W],
                        start=(rnd == 0), stop=(rnd == len(CHAINS[c]) - 1),
                        tile_position=(0, 32 * g))
            zcp[t % 2](gbuf[:, (t % 4) * N:(t % 4 + 1) * N], z_ps[:, 0:N])

        def stage2(t):
            g = t // GT
            if g not in ysb_tiles:
                ysb_tiles[g] = ypool.tile([P, GT * N], BF16, tag="ysb",
                                          name="ysb")
            ysb = ysb_tiles[g]
            if t % 2 == 0:
                ypair["t"] = psum_y.tile([P, 1024], F32, tag="yps",
                                         name="yps")
            y_ps = ypair["t"][:, 512 * (t % 2):512 * (t % 2) + N]
            rot2 = (t + 2) % 4      # matches the in-flight stage1 heavy slot
            for k in range(1, 5):
                g2 = (rot2 + k) % 4
                nc.tensor.matmul(
                    y_ps[32 * g2:32 * g2 + 32, :],
                    lhsT=w2[:, 32 * g2:32 * g2 + 32],
                    rhs=gbuf[:, (t % 4) * N:(t % 4 + 1) * N],
                    start=True, stop=True,
                    tile_position=(0, 32 * g2))
            tt = t % GT
            if t % 2 == 1:
                yp = ypair["t"][:].rearrange("p (s c) -> p s c", c=512)
                zcp[(t // 2 + 1) % 2](
                    ysb[:, (tt - 1) * N:(tt + 1) * N]
                        .rearrange("p (s c) -> p s c", c=N),
                    yp[:, :, 0:N])
            if g == NG - 1 and tt % 2 == 1:
                nc.sync.dma_start(
                    out=yf[:, (g * GT + tt - 1) * N:(g * GT + tt + 1) * N],
                    in_=ysb[:, (tt - 1) * N:(tt + 1) * N])
            elif tt == GT - 1:
                nc.sync.dma_start(
                    out=yf[:, g * GT * N:(g + 1) * GT * N], in_=ysb[:])
                del ysb_tiles[g]

        stage1(0)
        build_w2()
        stage1(1)
        stage1(2)
        for t in range(NT):
            if t + 3 < NT:
                stage1(t + 3)
            stage2(t)


def host_tables(wk, w_in, b_in, w_out):
    # H matrix: sums vector [T,CF,CL,RF,RL,c00,c0L,cL0,cLL] -> S[m], m=(dy,dx)
    Hm = np.zeros((9, 9), np.float32)
    Hm[0, :] = 1.0
    for m in range(9):
        dy, dx = divmod(m, 3)
        if dy == 0:
            Hm[4, m] -= 1.0
        if dy == 2:
            Hm[3, m] -= 1.0
        if dx == 0:
            Hm[2, m] -= 1.0
        if dx == 2:
            Hm[1, m] -= 1.0
    Hm[8, 0] = Hm[7, 2] = Hm[6, 6] = Hm[5, 8] = 1.0
    wk9 = wk.reshape(CIN, 9, 9).astype(np.float32) / float(H * W)  # [c, j, m]
    wkh = np.einsum("cjm,km->cjk", wk9, Hm).reshape(CIN, 81)
    wkh = np.tile(wkh, (BC, 1))

    lwin = np.kron(np.eye(BC, dtype=np.float32), w_in.T.astype(np.float32))
    brep = np.tile(b_in.astype(np.float32), BC)[:, None]
    w9 = w_out.reshape(COUT, CIN, 9).astype(np.float32)
    wo9 = np.concatenate(
        [np.kron(np.eye(BC, dtype=np.float32), w9[:, :, j].T) for j in range(9)],
        axis=1)
    m4 = np.zeros((P, 32), np.float32)
    m4[np.arange(P), np.arange(P) // CIN] = 1.0
    ident = np.eye(P, dtype=np.float32)
    return {
        "wkh": np.ascontiguousarray(wkh, np.float32),
        "lwin": np.ascontiguousarray(lwin).astype(ml_dtypes.bfloat16),
        "brep": np.ascontiguousarray(brep, np.float32),
        "wo9": np.ascontiguousarray(wo9).astype(ml_dtypes.bfloat16),
        "m4": np.ascontiguousarray(m4, np.float32),
        "ident": np.ascontiguousarray(ident).astype(ml_dtypes.bfloat16),
    }


_CACHE: dict = {}


def _get_program() -> bass.Bass:
    if "nc" not in _CACHE:
        nc = bacc.Bacc(
            trn_type="TRN2", target_bir_lowering=False, debug=False,
            num_devices=NCORES)
        build_program(nc)
        nc.compile()
        _CACHE["nc"] = nc
    return _CACHE["nc"]


def kernel(x, wk, w_in, b_in, w_out, _trace=False, _trace_kwargs=None):
    x = np.asarray(x, np.float32)
    xp = np.zeros((B, CIN, HP, WP), np.float32)
    xp[:, :, 1:H + 1, 1:W + 1] = x
    xpb = xp.astype(ml_dtypes.bfloat16).reshape(B, CIN, HP * WP)
    tables = host_tables(np.asarray(wk), np.asarray(w_in), np.asarray(b_in),
                         np.asarray(w_out))
    nc = _get_program()
    in_maps = [
        {"xpad": np.ascontiguousarray(
            xpb[c * BC:(c + 1) * BC].reshape(P, HP * WP)), **tables}
        for c in range(NCORES)
    ]
    res = run_bass_kernel_spmd(
        nc, in_maps, core_ids=list(range(NCORES)),
        trace=_trace, **(_trace_kwargs or {}))
    y = np.concatenate(
        [np.asarray(res.results[c]["y"]).astype(np.float32)
         for c in range(NCORES)], axis=0)
    if _trace:
        return y, res
    return y


if __name__ == "__main__":
    rng = np.random.default_rng(0)
    inputs = {
        "x": rng.standard_normal((B, CIN, H, W), np.float32),
        "wk": rng.standard_normal((CIN * 9, 1, 3, 3)).astype(np.float32) * 0.05,
        "w_in": rng.standard_normal((CIN, CIN)).astype(np.float32) * 0.05,
        "b_in": rng.standard_normal((CIN,)).astype(np.float32) * 0.05,
        "w_out": rng.standard_normal((COUT, CIN, 3, 3)).astype(np.float32) * 0.05,
    }
    y = kernel(**inputs)
    print("y", y.shape, y.dtype, float(np.abs(y).max()))


# revision 27
# speedup vs baseline: 1.0179x; 1.0179x over previous
"""Trainium2 Bass kernel for nn_BaseConvPlus (dense_cnn).

Math: the reference computes
  1) kernel[b,c,:,:]  = global-mean of a depthwise 3x3 conv of x          -> [B,CIN,3,3]
  2) win  = einsum(kernel, w_in) + b_in ; wout = einsum(kernel, w_out)
  3) y[b] = conv2d(x[b], weight[b]) with weight[b,o,i] = win[b,i]*wout[b,o]

Identities / tricks:
  * mean(conv(x, k)) over HxW only needs the total sum, edge-row/col sums
    and corner pixels of each channel (zero 'SAME' padding) - no conv.
    The tap-selection matrix is folded into the host-side wk tables, so
    kernel[b,c,j] = sum_k wkH[c,j,k] * sums[b,c,k] with sums = the 9
    reduced quantities [T, CF, CL, RF, RL, c00, c0L, cL0, cLL].
  * weight[b] is rank-1 across (o, i): y[b,o] = wout[b,o] * z[b] with
    z[b] = sum_i conv2d(x[b,i], win[b,i]).
  * x arrives host-padded (zero ring) and bf16, so all 9 (ky,kx) taps are
    plain shifted windows of the padded image.  Stage 1 runs the 9 tap
    matmuls (M=4 real outputs each) as 3 accumulation chains on 3
    concurrent PE *column groups* (tile_position col packing), so the 9
    image passes cost ~3.  One [96,384] eviction per tile feeds stage 2,
    a single K=96 matmul that applies wout (1 pass).  ~4 effective
    passes/tile vs 6 in the two-stage K=128/K=12 formulation, and the
    eviction count drops 1.5x.
  * bf16 end to end (input pre-cast on host, output upcast on host):
    halves both HBM phases.  Dummy matmuls paced by the input chunks keep
    the PE HAM warm (2.4 GHz) through the load phase.

Sharding: pure data parallel, 4 samples per core on 8 cores.
"""
import sys

sys.path.insert(0, "/opt/trn_rl_repo")

from contextlib import ExitStack

import ml_dtypes
import numpy as np

import concourse.bacc as bacc
import concourse.bass as bass
import concourse.mybir as mybir
import concourse.tile as tile
from concourse.bass_utils import run_bass_kernel_spmd

B, CIN, COUT, KS, H, W = 32, 32, 32, 3, 192, 192
NCORES = 8
BC = B // NCORES          # 4 samples per core
P = BC * CIN              # 128 partitions = (sample, channel)
WP = W + 2                # 194 padded cols
HP = H + 2                # 194 padded rows
R = 2                     # output rows per conv tile
NT = H // R               # 96 conv tiles
GT = 8                    # conv tiles per output DMA (16 rows)
NG = NT // GT             # 12 output DMAs
N = R * W                 # 384 moving columns per matmul
CHUNKS = [16] * 9 + [10] * 4 + [5, 5]   # input chunks over the 194 padded rows
NCHUNK = len(CHUNKS)      # 15
F32 = mybir.dt.float32
BF16 = mybir.dt.bfloat16
AX = mybir.AxisListType
OP = mybir.AluOpType


def build_program(nc: bass.Bass) -> None:
    x_d = nc.dram_tensor("xpad", [P, HP * WP], BF16, kind="ExternalInput").ap()
    wkh_d = nc.dram_tensor("wkh", [P, 81], F32, kind="ExternalInput").ap()
    lwin_d = nc.dram_tensor("lwin", [P, P], BF16, kind="ExternalInput").ap()
    brep_d = nc.dram_tensor("brep", [P, 1], F32, kind="ExternalInput").ap()
    wo9_d = nc.dram_tensor("wo9", [P, 9 * P], BF16, kind="ExternalInput").ap()
    m4_d = nc.dram_tensor("m4", [P, 32], F32, kind="ExternalInput").ap()
    ident_d = nc.dram_tensor("ident", [P, P], BF16, kind="ExternalInput").ap()
    y_d = nc.dram_tensor("y", [BC, COUT, H, W], BF16, kind="ExternalOutput").ap()

    yf = y_d.rearrange("b o h w -> (b o) (h w)")       # [128, 36864] bf16

    with tile.TileContext(nc) as tc, ExitStack() as ctx:
        const = ctx.enter_context(tc.tile_pool(name="const", bufs=1))
        ypool = ctx.enter_context(tc.tile_pool(name="ysb", bufs=3))
        psum_z = ctx.enter_context(tc.tile_pool(name="psum_z", bufs=4, space="PSUM"))
        psum_y = ctx.enter_context(tc.tile_pool(name="psum_y", bufs=2, space="PSUM"))

        xpad = const.tile([P, HP * WP], BF16)
        wkh = const.tile([P, 81], F32)
        lwin = const.tile([P, P], BF16)
        brep = const.tile([P, 1], F32)
        wo9 = const.tile([P, 9 * P], BF16)
        m4 = const.tile([P, 32], F32)
        ident = const.tile([P, P], BF16)
        scr = const.tile([P, 16 + 3 * NCHUNK], F32)
        t81 = const.tile([P, 81], F32)
        kern = const.tile([P, 9], F32)
        kernb = const.tile([P, 9], BF16)
        wS1 = const.tile([P, 9 * 4], BF16)     # stage1 lhsT per (ky,kx)
        w2 = const.tile([P, P], BF16)          # stage2 lhsT (wout block diag)
        e4 = const.tile([P, 4], BF16)
        gbuf = const.tile([P, 4 * N], BF16)    # 4 slots of stage1 output
        fold = const.tile([P, 2 * 1552], F32)  # Pool-engine chunk fold scratch
        rscr = const.tile([P, 16 * WP], BF16)  # ACT accum-reduce trash output

        x3 = xpad[:].rearrange("p (r c) -> p r c", c=WP)   # [128, 194, 194]

        nc.vector.memset(w2[:], 0.0)

        # PE keep-warm target (never read; WAW chain keeps it serialized).
        # Shares the psum_z pool: conv stage1 tiles recycle these buffers.
        pdum_t = psum_z.tile([P, 512], F32, tag="zps", name="pdum")
        pdum = pdum_t[:, 0:512]

        # ---- input phase: chunked DMA + running sums + PE warmup ----
        r0 = 0
        for i, lr in enumerate(CHUNKS):
            sl = xpad[:, r0 * WP:(r0 + lr) * WP]
            nc.sync.dma_start(out=sl, in_=x_d[:, r0 * WP:(r0 + lr) * WP])
            if i == 0:
                # constants on the same HWDGE queue (spread over all 16 DMA
                # engines; the SWDGE queue would serialize them on engine 15
                # and that engine's share of every input chunk behind them)
                nc.sync.dma_start(out=wkh[:], in_=wkh_d)
                nc.sync.dma_start(out=lwin[:], in_=lwin_d)
                nc.sync.dma_start(out=brep[:], in_=brep_d)
                nc.sync.dma_start(out=wo9[:], in_=wo9_d)
                nc.sync.dma_start(out=m4[:], in_=m4_d)
                nc.sync.dma_start(out=ident[:], in_=ident_d)
            s3 = sl.rearrange("p (r c) -> p r c", c=WP)
            # even chunks: Pool folds in half, DVE reduces the folded half
            # (Pool is slow: ~0.6 elem/cycle, so only DVE's share is folded);
            # odd chunks: ACT accum-reduce directly
            if i % 2 == 0 and i != NCHUNK - 1:
                hl = lr * WP // 2
                fsl = fold[:, (i // 2 % 2) * 1552:(i // 2 % 2) * 1552 + hl]
                nc.gpsimd.tensor_add(fsl, sl[:, 0:hl], sl[:, hl:2 * hl])
                nc.vector.reduce_sum(out=scr[:, 16 + i:17 + i], in_=fsl,
                                     axis=AX.X)
            elif i % 2 == 0:
                nc.vector.reduce_sum(out=scr[:, 16 + i:17 + i], in_=sl,
                                     axis=AX.X)
            else:
                nc.scalar.activation(
                    out=rscr[:, 0:lr * WP], in_=sl,
                    func=mybir.ActivationFunctionType.Copy,
                    accum_out=scr[:, 16 + i:17 + i])
            nc.vector.reduce_sum(
                out=scr[:, 16 + NCHUNK + i:17 + NCHUNK + i],
                in_=s3[:, :, 1], axis=AX.X)
            nc.vector.reduce_sum(
                out=scr[:, 16 + 2 * NCHUNK + i:17 + 2 * NCHUNK + i],
                in_=s3[:, :, W], axis=AX.X)
            if i == 0:     # first data row + top corners (chunk 0 only)
                nc.vector.reduce_sum(out=scr[:, 3:4], in_=x3[:, 1, :],
                                     axis=AX.X)
                nc.vector.tensor_copy(
                    scr[:, 5:7], xpad[:, WP + 1:WP + W + 1:W - 1])
            # HAM keep-warm: dummy matmuls paced by this chunk's arrival
            ndum = 6 if i < 11 else (3 if i < 13 else 2)
            for k in range(ndum):
                nc.tensor.matmul(
                    pdum[:], lhsT=xpad[:, 0:128],
                    rhs=xpad[:, r0 * WP:r0 * WP + 512],
                    start=True, stop=True)
            r0 += lr

        # remaining edge sums + grouped T/CF/CL partial reduction
        nc.vector.reduce_sum(out=scr[:, 4:5], in_=x3[:, H, :], axis=AX.X)
        nc.vector.tensor_copy(
            scr[:, 7:9], xpad[:, H * WP + 1:H * WP + W + 1:W - 1])
        nc.vector.reduce_sum(
            out=scr[:, 0:3],
            in_=scr[:, 16:16 + 3 * NCHUNK].rearrange("p (g i) -> p g i", g=3),
            axis=AX.X)

        # PE bridge through the weight-derivation chain (no deps: these run
        # back-to-back the moment the paced dummies drain, keeping HAM warm)
        for k in range(12):
            nc.tensor.matmul(pdum[:], lhsT=xpad[:, 0:128],
                             rhs=xpad[:, 0:512], start=True, stop=True)

        # kernel[p, j] = sum_k wkH[p, j*9+k] * sums[p, k]
        sums9 = scr[:, 0:9].unsqueeze(1).broadcast_to([P, 9, 9])
        nc.vector.tensor_mul(
            t81[:].rearrange("p (j m) -> p j m", m=9),
            wkh[:].rearrange("p (j m) -> p j m", m=9), sums9)
        nc.vector.reduce_sum(
            out=kern[:], in_=t81[:].rearrange("p (j m) -> p j m", m=9),
            axis=AX.X)
        nc.vector.tensor_copy(kernb[:], kern[:])

        # weight-build outputs live in psum_z-pool buffers (recycled by
        # the first conv tiles once these reads complete); boot garbage in
        # the never-matmul-written regions is zeroed once
        small = psum_z.tile([P, 512], F32, tag="zps", name="small")
        nc.vector.memset(small[:, 160:512], 0.0)
        zscr1 = psum_z.tile([P, 512], F32, tag="zps", name="zscr1")
        nc.vector.memset(zscr1[:], 0.0)
        zscr2 = psum_z.tile([P, 512], F32, tag="zps", name="zscr2")
        nc.vector.memset(zscr2[:], 0.0)
        win_ps = small[:, 0:9]
        wout_ps = small[:, 9:10]
        et_ps = small[0:4, 32:96].bitcast(BF16)

        # win = blockdiag(w_in.T) @ kernel  (+ b_in added in the stt below)
        nc.tensor.matmul(win_ps, lhsT=lwin[:], rhs=kernb[:],
                         start=True, stop=True)

        # stage1 weights: wS1[(b,i), (j,b')] = (win[b,i,j]+b_in[i]) d(b,b')
        wv = win_ps.unsqueeze(2).broadcast_to([P, 9, 4])
        mv = m4[:, 0:4].unsqueeze(1).broadcast_to([P, 9, 4])
        nc.vector.scalar_tensor_tensor(
            wS1[:].rearrange("p (j c) -> p j c", c=4),
            wv, brep[:], mv, op0=OP.add, op1=OP.mult)

        # wout[(b,o)] = sum_j blockdiag(w_out[:,:,j].T) @ kernel[:, j]
        for j in range(9):
            nc.tensor.matmul(
                wout_ps, lhsT=wo9[:, j * P:(j + 1) * P],
                rhs=kernb[:, j:j + 1], start=(j == 0), stop=(j == 8))
        # W2[32g+b, (b',o)] = wout[(b',o)] d(b,b') for g=0,1,2
        nc.vector.tensor_scalar_mul(e4[:], m4[:, 0:4], wout_ps)
        nc.tensor.transpose(et_ps, e4[:], ident[:])
        for g in range(4):
            nc.vector.tensor_copy(w2[32 * g:32 * g + 4, :], et_ps)

        # ---- conv phase ----
        # (gpsimd/Pool cannot read PSUM: evictions alternate DVE <-> ACT)
        ysb_tiles = {}
        ypair = {}
        zcp = [nc.vector.tensor_copy,
               lambda o, i_: nc.scalar.copy(out=o, in_=i_)]

        # 9 tap chains spread (3,2,2,2) over the 4 PE column groups; the
        # heavy slot rotates with t so sustained load is 13/4 streams/group
        CHAINS = [[(0, 0), (1, 0), (2, 0)], [(0, 1), (1, 1)],
                  [(2, 1), (0, 2)], [(1, 2), (2, 2)]]

        def stage1(t):
            rot = t % 4
            z_ps = psum_z.tile([P, 512], F32, tag="zps", name="zps")
            for rnd in range(3):
                for c in (1, 2, 3, 0):
                    if rnd >= len(CHAINS[c]):
                        continue
                    ky, kx = CHAINS[c][rnd]
                    g = (c + rot) % 4
                    nc.tensor.matmul(
                        z_ps[32 * g:32 * g + 4, 0:N],
                        lhsT=wS1[:, (3 * ky + kx) * 4:(3 * ky + kx + 1) * 4],
                        rhs=x3[:, 2 * t + ky:2 * t + ky + 2, kx:kx + W],
                        start=(rnd == 0), stop=(rnd == len(CHAINS[c]) - 1),
                        tile_position=(0, 32 * g))
            zcp[t % 2](gbuf[:, (t % 4) * N:(t % 4 + 1) * N], z_ps[:, 0:N])

        def stage2(t):
            g = t // GT
            if g not in ysb_tiles:
                ysb_tiles[g] = ypool.tile([P, GT * N], BF16, tag="ysb",
                                          name="ysb")
            ysb = ysb_tiles[g]
            if t % 2 == 0:
                ypair["t"] = psum_y.tile([P, 1024], F32, tag="yps",
                                         name="yps")
            y_ps = ypair["t"][:, 512 * (t % 2):512 * (t % 2) + N]
            rot2 = (t + 2) % 4      # matches the in-flight stage1 heavy slot
            for k in range(1, 5):
                g2 = (rot2 + k) % 4
                nc.tensor.matmul(
                    y_ps[32 * g2:32 * g2 + 32, :],
                    lhsT=w2[:, 32 * g2:32 * g2 + 32],
                    rhs=gbuf[:, (t % 4) * N:(t % 4 + 1) * N],
                    start=True, stop=True,
                    tile_position=(0, 32 * g2))
            tt = t % GT
            if t % 2 == 1:
                yp = ypair["t"][:].rearrange("p (s c) -> p s c", c=512)
                zcp[(t // 2 + 1) % 2](
                    ysb[:, (tt - 1) * N:(tt + 1) * N]
                        .rearrange("p (s c) -> p s c", c=N),
                    yp[:, :, 0:N])
            if g == NG - 1 and tt % 2 == 1:
                nc.sync.dma_start(
                    out=yf[:, (g * GT + tt - 1) * N:(g * GT + tt + 1) * N],
                    in_=ysb[:, (tt - 1) * N:(tt + 1) * N])
            elif tt == GT - 1:
                nc.sync.dma_start(
                    out=yf[:, g * GT * N:(g + 1) * GT * N], in_=ysb[:])
                del ysb_tiles[g]

        stage1(0)
        stage1(1)
        stage1(2)
        for t in range(NT):
            if t + 3 < NT:
                stage1(t + 3)
            stage2(t)


def host_tables(wk, w_in, b_in, w_out):
    # H matrix: sums vector [T,CF,CL,RF,RL,c00,c0L,cL0,cLL] -> S[m], m=(dy,dx)
    Hm = np.zeros((9, 9), np.float32)
    Hm[0, :] = 1.0
    for m in range(9):
        dy, dx = divmod(m, 3)
        if dy == 0:
            Hm[4, m] -= 1.0
        if dy == 2:
            Hm[3, m] -= 1.0
        if dx == 0:
            Hm[2, m] -= 1.0
        if dx == 2:
            Hm[1, m] -= 1.0
    Hm[8, 0] = Hm[7, 2] = Hm[6, 6] = Hm[5, 8] = 1.0
    wk9 = wk.reshape(CIN, 9, 9).astype(np.float32) / float(H * W)  # [c, j, m]
    wkh = np.einsum("cjm,km->cjk", wk9, Hm).reshape(CIN, 81)
    wkh = np.tile(wkh, (BC, 1))

    lwin = np.kron(np.eye(BC, dtype=np.float32), w_in.T.astype(np.float32))
    brep = np.tile(b_in.astype(np.float32), BC)[:, None]
    w9 = w_out.reshape(COUT, CIN, 9).astype(np.float32)
    wo9 = np.concatenate(
        [np.kron(np.eye(BC, dtype=np.float32), w9[:, :, j].T) for j in range(9)],
        axis=1)
    m4 = np.zeros((P, 32), np.float32)
    m4[np.arange(P), np.arange(P) // CIN] = 1.0
    ident = np.eye(P, dtype=np.float32)
    return {
        "wkh": np.ascontiguousarray(wkh, np.float32),
        "lwin": np.ascontiguousarray(lwin).astype(ml_dtypes.bfloat16),
        "brep": np.ascontiguousarray(brep, np.float32),
        "wo9": np.ascontiguousarray(wo9).astype(ml_dtypes.bfloat16),
        "m4": np.ascontiguousarray(m4, np.float32),
        "ident": np.ascontiguousarray(ident).astype(ml_dtypes.bfloat16),
    }


_CACHE: dict = {}


def _get_program() -> bass.Bass:
    if "nc" not in _CACHE:
        nc = bacc.Bacc(
            trn_type="TRN2", target_bir_lowering=False, debug=False,
            num_devices=NCORES)
        build_program(nc)
        nc.compile()
        _CACHE["nc"] = nc
    return _CACHE["nc"]


def kernel(x, wk, w_in, b_in, w_out, _trace=False, _trace_kwargs=None):
    x = np.asarray(x, np.float32)
    xp = np.zeros((B, CIN, HP, WP), np.float32)
    xp[:, :, 1:H + 1, 1:W + 1] = x
    xpb = xp.astype(ml_dtypes.bfloat16).reshape(B, CIN, HP * WP)
    tables = host_tables(np.asarray(wk), np.asarray(w_in), np.asarray(b_in),
                         np.asarray(w_out))
    nc = _get_program()
    in_maps = [
        {"xpad": np.ascontiguousarray(
            xpb[c * BC:(c + 1) * BC].reshape(P, HP * WP)), **tables}
        for c in range(NCORES)
    ]
    res = run_bass_kernel_spmd(
        nc, in_maps, core_ids=list(range(NCORES)),
        trace=_trace, **(_trace_kwargs or {}))
    y = np.concatenate(
        [np.asarray(res.results[c]["y"]).astype(np.float32)
         for c in range(NCORES)], axis=0)
    if _trace:
        return y, res
    return y


if __name__ == "__main__":
    rng = np.random.default_rng(0)
    inputs = {
        "x": rng.standard_normal((B, CIN, H, W), np.float32),
        "wk": rng.standard_normal((CIN * 9, 1, 3, 3)).astype(np.float32) * 0.05,
        "w_in": rng.standard_normal((CIN, CIN)).astype(np.float32) * 0.05,
        "b_in": rng.standard_normal((CIN,)).astype(np.float32) * 0.05,
        "w_out": rng.standard_normal((COUT, CIN, 3, 3)).astype(np.float32) * 0.05,
    }
    y = kernel(**inputs)
    print("y", y.shape, y.dtype, float(np.abs(y).max()))


# revision 28
# speedup vs baseline: 1.0356x; 1.0174x over previous
"""Trainium2 Bass kernel for nn_BaseConvPlus (dense_cnn).

Math: the reference computes
  1) kernel[b,c,:,:]  = global-mean of a depthwise 3x3 conv of x          -> [B,CIN,3,3]
  2) win  = einsum(kernel, w_in) + b_in ; wout = einsum(kernel, w_out)
  3) y[b] = conv2d(x[b], weight[b]) with weight[b,o,i] = win[b,i]*wout[b,o]

Identities / tricks:
  * mean(conv(x, k)) over HxW only needs the total sum, edge-row/col sums
    and corner pixels of each channel (zero 'SAME' padding) - no conv.
    The tap-selection matrix is folded into the host-side wk tables, so
    kernel[b,c,j] = sum_k wkH[c,j,k] * sums[b,c,k] with sums = the 9
    reduced quantities [T, CF, CL, RF, RL, c00, c0L, cL0, cLL].
  * weight[b] is rank-1 across (o, i): y[b,o] = wout[b,o] * z[b] with
    z[b] = sum_i conv2d(x[b,i], win[b,i]).
  * x arrives host-padded (zero ring) and bf16, so all 9 (ky,kx) taps are
    plain shifted windows of the padded image.  Stage 1 runs the 9 tap
    matmuls (M=4 real outputs each) as four accumulation chains (3,2,2,2)
    on the 4 PE *column groups* (tile_position col packing), heavy slot
    rotating with t; stage 2 applies wout as 4 col-tiled K=128 matmuls on
    the evicted z.  Sustained PE cost ~13/4 streams/tile vs 6 full passes
    in the two-stage K=128/K=12 formulation.
  * Evictions are the co-bottleneck: z leaves per tile ([128,384],
    single-bank psum tiles, bufs=4 so the WAR recycle chain stays off the
    critical path) and y per tile-pair (two-bank psum tiles, one strided
    copy), alternating DVE <-> ACT.
  * bf16 end to end (input pre-cast on host, output upcast on host):
    halves both HBM phases.  Input-chunk row sums are split DVE/ACT (ACT
    via activation-accumulate), with the Pool engine pre-folding DVE's
    share; the global-mean -> weights chain is the only hard sync point
    between the load and conv phases.  Dummy matmuls paced by the input
    chunks keep the PE HAM warm (2.4 GHz) through the load phase.

Sharding: pure data parallel, 4 samples per core on 8 cores.
"""
import sys

sys.path.insert(0, "/opt/trn_rl_repo")

from contextlib import ExitStack

import ml_dtypes
import numpy as np

import concourse.bacc as bacc
import concourse.bass as bass
import concourse.mybir as mybir
import concourse.tile as tile
from concourse.bass_utils import run_bass_kernel_spmd

B, CIN, COUT, KS, H, W = 32, 32, 32, 3, 192, 192
NCORES = 8
BC = B // NCORES          # 4 samples per core
P = BC * CIN              # 128 partitions = (sample, channel)
WP = W + 2                # 194 padded cols
HP = H + 2                # 194 padded rows
R = 2                     # output rows per conv tile
NT = H // R               # 96 conv tiles
GT = 8                    # conv tiles per output DMA (16 rows)
NG = NT // GT             # 12 output DMAs
N = R * W                 # 384 moving columns per matmul
CHUNKS = [16] * 9 + [10] * 4 + [5, 5]   # input chunks over the 194 padded rows
NCHUNK = len(CHUNKS)      # 15
F32 = mybir.dt.float32
BF16 = mybir.dt.bfloat16
AX = mybir.AxisListType
OP = mybir.AluOpType


def build_program(nc: bass.Bass) -> None:
    x_d = nc.dram_tensor("xpad", [P, HP * WP], BF16, kind="ExternalInput").ap()
    wkh_d = nc.dram_tensor("wkh", [P, 81], F32, kind="ExternalInput").ap()
    lwin_d = nc.dram_tensor("lwin", [P, P], BF16, kind="ExternalInput").ap()
    brep_d = nc.dram_tensor("brep", [P, 1], F32, kind="ExternalInput").ap()
    wo9_d = nc.dram_tensor("wo9", [P, 9 * P], BF16, kind="ExternalInput").ap()
    m4_d = nc.dram_tensor("m4", [P, 32], F32, kind="ExternalInput").ap()
    ident_d = nc.dram_tensor("ident", [P, P], BF16, kind="ExternalInput").ap()
    y_d = nc.dram_tensor("y", [BC, COUT, H, W], BF16, kind="ExternalOutput").ap()

    yf = y_d.rearrange("b o h w -> (b o) (h w)")       # [128, 36864] bf16

    with tile.TileContext(nc) as tc, ExitStack() as ctx:
        const = ctx.enter_context(tc.tile_pool(name="const", bufs=1))
        ypool = ctx.enter_context(tc.tile_pool(name="ysb", bufs=3))
        psum_z = ctx.enter_context(tc.tile_pool(name="psum_z", bufs=4, space="PSUM"))
        psum_y = ctx.enter_context(tc.tile_pool(name="psum_y", bufs=2, space="PSUM"))

        xpad = const.tile([P, HP * WP], BF16)
        wkh = const.tile([P, 81], F32)
        lwin = const.tile([P, P], BF16)
        brep = const.tile([P, 1], F32)
        wo9 = const.tile([P, 9 * P], BF16)
        m4 = const.tile([P, 32], F32)
        ident = const.tile([P, P], BF16)
        scr = const.tile([P, 16 + 3 * NCHUNK], F32)
        t81 = const.tile([P, 81], F32)
        kern = const.tile([P, 9], F32)
        kernb = const.tile([P, 9], BF16)
        wS1 = const.tile([P, 9 * 4], BF16)     # stage1 lhsT per (ky,kx)
        w2 = const.tile([P, P], BF16)          # stage2 lhsT (wout block diag)
        e4 = const.tile([P, 4], BF16)
        gbuf = const.tile([P, 4 * N], BF16)    # 4 slots of stage1 output
        fold = const.tile([P, 2 * 1552], F32)  # Pool-engine chunk fold scratch
        rscr = const.tile([P, 16 * WP], BF16)  # ACT accum-reduce trash output

        x3 = xpad[:].rearrange("p (r c) -> p r c", c=WP)   # [128, 194, 194]

        nc.vector.memset(w2[:], 0.0)

        # PE keep-warm target (never read; WAW chain keeps it serialized).
        # Shares the psum_z pool: conv stage1 tiles recycle these buffers.
        pdum_t = psum_z.tile([P, 512], F32, tag="zps", name="pdum")
        pdum = pdum_t[:, 0:512]

        # ---- input phase: chunked DMA + running sums + PE warmup ----
        r0 = 0
        for i, lr in enumerate(CHUNKS):
            sl = xpad[:, r0 * WP:(r0 + lr) * WP]
            nc.sync.dma_start(out=sl, in_=x_d[:, r0 * WP:(r0 + lr) * WP])
            if i == 0:
                # constants on the same HWDGE queue (spread over all 16 DMA
                # engines; the SWDGE queue would serialize them on engine 15
                # and that engine's share of every input chunk behind them)
                nc.sync.dma_start(out=wkh[:], in_=wkh_d)
                nc.sync.dma_start(out=lwin[:], in_=lwin_d)
                nc.sync.dma_start(out=brep[:], in_=brep_d)
                nc.sync.dma_start(out=wo9[:], in_=wo9_d)
                nc.sync.dma_start(out=m4[:], in_=m4_d)
                nc.sync.dma_start(out=ident[:], in_=ident_d)
            s3 = sl.rearrange("p (r c) -> p r c", c=WP)
            # even chunks: Pool folds in half, DVE reduces the folded half
            # (Pool is slow: ~0.6 elem/cycle, so only DVE's share is folded);
            # odd chunks: ACT accum-reduce directly
            if i % 2 == 0 and i != NCHUNK - 1:
                hl = lr * WP // 2
                fsl = fold[:, (i // 2 % 2) * 1552:(i // 2 % 2) * 1552 + hl]
                nc.gpsimd.tensor_add(fsl, sl[:, 0:hl], sl[:, hl:2 * hl])
                nc.vector.reduce_sum(out=scr[:, 16 + i:17 + i], in_=fsl,
                                     axis=AX.X)
            elif i % 2 == 0:
                nc.vector.reduce_sum(out=scr[:, 16 + i:17 + i], in_=sl,
                                     axis=AX.X)
            else:
                nc.scalar.activation(
                    out=rscr[:, 0:lr * WP], in_=sl,
                    func=mybir.ActivationFunctionType.Copy,
                    accum_out=scr[:, 16 + i:17 + i])
            nc.vector.reduce_sum(
                out=scr[:, 16 + NCHUNK + i:17 + NCHUNK + i],
                in_=s3[:, :, 1], axis=AX.X)
            nc.vector.reduce_sum(
                out=scr[:, 16 + 2 * NCHUNK + i:17 + 2 * NCHUNK + i],
                in_=s3[:, :, W], axis=AX.X)
            if i == 0:     # first data row + top corners (chunk 0 only)
                nc.vector.reduce_sum(out=scr[:, 3:4], in_=x3[:, 1, :],
                                     axis=AX.X)
                nc.vector.tensor_copy(
                    scr[:, 5:7], xpad[:, WP + 1:WP + W + 1:W - 1])
            # HAM keep-warm: dummy matmuls paced by this chunk's arrival
            ndum = 6 if i < 11 else (3 if i < 13 else 2)
            for k in range(ndum):
                nc.tensor.matmul(
                    pdum[:], lhsT=xpad[:, 0:128],
                    rhs=xpad[:, r0 * WP:r0 * WP + 512],
                    start=True, stop=True)
            r0 += lr

        # remaining edge sums + grouped T/CF/CL partial reduction
        nc.vector.reduce_sum(out=scr[:, 4:5], in_=x3[:, H, :], axis=AX.X)
        nc.vector.tensor_copy(
            scr[:, 7:9], xpad[:, H * WP + 1:H * WP + W + 1:W - 1])
        nc.vector.reduce_sum(
            out=scr[:, 0:3],
            in_=scr[:, 16:16 + 3 * NCHUNK].rearrange("p (g i) -> p g i", g=3),
            axis=AX.X)

        # PE bridge through the weight-derivation chain (no deps: these run
        # back-to-back the moment the paced dummies drain, keeping HAM warm)
        for k in range(12):
            nc.tensor.matmul(pdum[:], lhsT=xpad[:, 0:128],
                             rhs=xpad[:, 0:512], start=True, stop=True)

        # kernel[p, j] = sum_k wkH[p, j*9+k] * sums[p, k]
        sums9 = scr[:, 0:9].unsqueeze(1).broadcast_to([P, 9, 9])
        nc.vector.tensor_mul(
            t81[:].rearrange("p (j m) -> p j m", m=9),
            wkh[:].rearrange("p (j m) -> p j m", m=9), sums9)
        nc.vector.reduce_sum(
            out=kern[:], in_=t81[:].rearrange("p (j m) -> p j m", m=9),
            axis=AX.X)
        nc.vector.tensor_copy(kernb[:], kern[:])

        # weight-build outputs live in psum_z-pool buffers (recycled by
        # the first conv tiles once these reads complete); boot garbage in
        # the never-matmul-written regions is zeroed once
        small = psum_z.tile([P, 512], F32, tag="zps", name="small")
        nc.vector.memset(small[:, 160:512], 0.0)
        zscr1 = psum_z.tile([P, 512], F32, tag="zps", name="zscr1")
        nc.vector.memset(zscr1[:], 0.0)
        zscr2 = psum_z.tile([P, 512], F32, tag="zps", name="zscr2")
        nc.vector.memset(zscr2[:], 0.0)
        win_ps = small[:, 0:9]
        wout_ps = small[:, 9:10]
        et_ps = small[0:4, 32:96].bitcast(BF16)

        # win = blockdiag(w_in.T) @ kernel  (+ b_in added in the stt below)
        nc.tensor.matmul(win_ps, lhsT=lwin[:], rhs=kernb[:],
                         start=True, stop=True)

        # stage1 weights: wS1[(b,i), (j,b')] = (win[b,i,j]+b_in[i]) d(b,b')
        wv = win_ps.unsqueeze(2).broadcast_to([P, 9, 4])
        mv = m4[:, 0:4].unsqueeze(1).broadcast_to([P, 9, 4])
        nc.vector.scalar_tensor_tensor(
            wS1[:].rearrange("p (j c) -> p j c", c=4),
            wv, brep[:], mv, op0=OP.add, op1=OP.mult)

        # wout[(b,o)] = sum_j blockdiag(w_out[:,:,j].T) @ kernel[:, j]
        for j in range(9):
            nc.tensor.matmul(
                wout_ps, lhsT=wo9[:, j * P:(j + 1) * P],
                rhs=kernb[:, j:j + 1], start=(j == 0), stop=(j == 8))
        # W2[32g+b, (b',o)] = wout[(b',o)] d(b,b') for g=0,1,2
        nc.vector.tensor_scalar_mul(e4[:], m4[:, 0:4], wout_ps)
        nc.tensor.transpose(et_ps, e4[:], ident[:])
        for g in range(4):
            nc.vector.tensor_copy(w2[32 * g:32 * g + 4, :], et_ps)

        # ---- conv phase ----
        # (gpsimd/Pool cannot read PSUM: evictions alternate DVE <-> ACT)
        ysb_tiles = {}
        ypair = {}
        zcp = [nc.vector.tensor_copy,
               lambda o, i_: nc.scalar.copy(out=o, in_=i_)]

        # 9 tap chains spread (3,2,2,2) over the 4 PE column groups; the
        # heavy slot rotates with t so sustained load is 13/4 streams/group
        CHAINS = [[(0, 0), (1, 0), (2, 0)], [(0, 1), (1, 1)],
                  [(2, 1), (0, 2)], [(1, 2), (2, 2)]]

        def stage1(t):
            rot = t % 4
            z_ps = psum_z.tile([P, 512], F32, tag="zps", name="zps")
            for rnd in range(3):
                for c in (1, 2, 3, 0):
                    if rnd >= len(CHAINS[c]):
                        continue
                    ky, kx = CHAINS[c][rnd]
                    g = (c + rot) % 4
                    nc.tensor.matmul(
                        z_ps[32 * g:32 * g + 4, 0:N],
                        lhsT=wS1[:, (3 * ky + kx) * 4:(3 * ky + kx + 1) * 4],
                        rhs=x3[:, 2 * t + ky:2 * t + ky + 2, kx:kx + W],
                        start=(rnd == 0), stop=(rnd == len(CHAINS[c]) - 1),
                        tile_position=(0, 32 * g))
            zcp[t % 2](gbuf[:, (t % 4) * N:(t % 4 + 1) * N], z_ps[:, 0:N])

        def stage2(t):
            g = t // GT
            if g not in ysb_tiles:
                ysb_tiles[g] = ypool.tile([P, GT * N], BF16, tag="ysb",
                                          name="ysb")
            ysb = ysb_tiles[g]
            if t % 2 == 0:
                ypair["t"] = psum_y.tile([P, 1024], F32, tag="yps",
                                         name="yps")
            y_ps = ypair["t"][:, 512 * (t % 2):512 * (t % 2) + N]
            rot2 = (t + 2) % 4      # matches the in-flight stage1 heavy slot
            for k in range(1, 5):
                g2 = (rot2 + k) % 4
                nc.tensor.matmul(
                    y_ps[32 * g2:32 * g2 + 32, :],
                    lhsT=w2[:, 32 * g2:32 * g2 + 32],
                    rhs=gbuf[:, (t % 4) * N:(t % 4 + 1) * N],
                    start=True, stop=True,
                    tile_position=(0, 32 * g2))
            tt = t % GT
            if t % 2 == 1:
                yp = ypair["t"][:].rearrange("p (s c) -> p s c", c=512)
                zcp[(t // 2 + 1) % 2](
                    ysb[:, (tt - 1) * N:(tt + 1) * N]
                        .rearrange("p (s c) -> p s c", c=N),
                    yp[:, :, 0:N])
            if g == NG - 1 and tt % 2 == 1:
                nc.sync.dma_start(
                    out=yf[:, (g * GT + tt - 1) * N:(g * GT + tt + 1) * N],
                    in_=ysb[:, (tt - 1) * N:(tt + 1) * N])
            elif tt == GT - 1:
                nc.sync.dma_start(
                    out=yf[:, g * GT * N:(g + 1) * GT * N], in_=ysb[:])
                del ysb_tiles[g]

        stage1(0)
        stage1(1)
        stage1(2)
        for t in range(NT):
            if t + 3 < NT:
                stage1(t + 3)
            stage2(t)


def host_tables(wk, w_in, b_in, w_out):
    # H matrix: sums vector [T,CF,CL,RF,RL,c00,c0L,cL0,cLL] -> S[m], m=(dy,dx)
    Hm = np.zeros((9, 9), np.float32)
    Hm[0, :] = 1.0
    for m in range(9):
        dy, dx = divmod(m, 3)
        if dy == 0:
            Hm[4, m] -= 1.0
        if dy == 2:
            Hm[3, m] -= 1.0
        if dx == 0:
            Hm[2, m] -= 1.0
        if dx == 2:
            Hm[1, m] -= 1.0
    Hm[8, 0] = Hm[7, 2] = Hm[6, 6] = Hm[5, 8] = 1.0
    wk9 = wk.reshape(CIN, 9, 9).astype(np.float32) / float(H * W)  # [c, j, m]
    wkh = np.einsum("cjm,km->cjk", wk9, Hm).reshape(CIN, 81)
    wkh = np.tile(wkh, (BC, 1))

    lwin = np.kron(np.eye(BC, dtype=np.float32), w_in.T.astype(np.float32))
    brep = np.tile(b_in.astype(np.float32), BC)[:, None]
    w9 = w_out.reshape(COUT, CIN, 9).astype(np.float32)
    wo9 = np.concatenate(
        [np.kron(np.eye(BC, dtype=np.float32), w9[:, :, j].T) for j in range(9)],
        axis=1)
    m4 = np.zeros((P, 32), np.float32)
    m4[np.arange(P), np.arange(P) // CIN] = 1.0
    ident = np.eye(P, dtype=np.float32)
    return {
        "wkh": np.ascontiguousarray(wkh, np.float32),
        "lwin": np.ascontiguousarray(lwin).astype(ml_dtypes.bfloat16),
        "brep": np.ascontiguousarray(brep, np.float32),
        "wo9": np.ascontiguousarray(wo9).astype(ml_dtypes.bfloat16),
        "m4": np.ascontiguousarray(m4, np.float32),
        "ident": np.ascontiguousarray(ident).astype(ml_dtypes.bfloat16),
    }


_CACHE: dict = {}


def _get_program() -> bass.Bass:
    if "nc" not in _CACHE:
        nc = bacc.Bacc(
            trn_type="TRN2", target_bir_lowering=False, debug=False,
            num_devices=NCORES)
        build_program(nc)
        nc.compile()
        _CACHE["nc"] = nc
    return _CACHE["nc"]


def kernel(x, wk, w_in, b_in, w_out, _trace=False, _trace_kwargs=None):
    x = np.asarray(x, np.float32)
    xp = np.zeros((B, CIN, HP, WP), np.float32)
    xp[:, :, 1:H + 1, 1:W + 1] = x
    xpb = xp.astype(ml_dtypes.bfloat16).reshape(B, CIN, HP * WP)
    tables = host_tables(np.asarray(wk), np.asarray(w_in), np.asarray(b_in),
                         np.asarray(w_out))
    nc = _get_program()
    in_maps = [
        {"xpad": np.ascontiguousarray(
            xpb[c * BC:(c + 1) * BC].reshape(P, HP * WP)), **tables}
        for c in range(NCORES)
    ]
    res = run_bass_kernel_spmd(
        nc, in_maps, core_ids=list(range(NCORES)),
        trace=_trace, **(_trace_kwargs or {}))
    y = np.concatenate(
        [np.asarray(res.results[c]["y"]).astype(np.float32)
         for c in range(NCORES)], axis=0)
    if _trace:
        return y, res
    return y


if __name__ == "__main__":
    rng = np.random.default_rng(0)
    inputs = {
        "x": rng.standard_normal((B, CIN, H, W), np.float32),
        "wk": rng.standard_normal((CIN * 9, 1, 3, 3)).astype(np.float32) * 0.05,
        "w_in": rng.standard_normal((CIN, CIN)).astype(np.float32) * 0.05,
        "b_in": rng.standard_normal((CIN,)).astype(np.float32) * 0.05,
        "w_out": rng.standard_normal((COUT, CIN, 3, 3)).astype(np.float32) * 0.05,
    }
    y = kernel(**inputs)
    print("y", y.shape, y.dtype, float(np.abs(y).max()))
